# revision 2
# baseline (speedup 1.0000x reference)
"""MeanAggregator (GNN mean message passing) on 8 Trainium2 NeuronCores.

reference:
    neigh_feats = features[neigh_idx]          # [batch, num_sample, d_feat]
    out = mean(neigh_feats, axis=1)            # [batch, d_feat]

Shapes (hardcoded): features [1_000_000, 128] f32, neigh_idx [100_000, 16] i64.

Strategy: data-parallel over the batch across 8 cores (12_500 rows each),
features replicated. Per core: per-partition indirect-DMA gathers (one
512 B feature row per partition per DMA — the only indirect form the SWDGE
ucode supports), vector-engine tree reduction over the 16 neighbors, scale
by 1/16, DMA out. Memory-bound: ~102 MB gathered + 6.4 MB written per core.

Layout per core: batch row b -> partition b % 128, slot b // 128.
Tile j covers batch rows [j*128, (j+1)*128): its 16 gathers (one per
neighbor sample) each fetch one row per partition into g[p, s*128:(s+1)*128].
"""

import numpy as np

import concourse.bacc as bacc
import concourse.bass as bass
import concourse.mybir as mybir
import concourse.tile as tile
from concourse.bass_utils import run_bass_kernel_spmd

N_CORES = 8
D = 128          # feature dim
S = 16           # neighbors per node
NUM_NODES = 1_000_000
BATCH = 100_000
B_CORE = BATCH // N_CORES   # 12_500
P = 128
NTILES = (B_CORE + P - 1) // P   # 98 (last tile ragged: 12500 = 97*128 + 84)


def build_nc(num_nodes=NUM_NODES, b_core=B_CORE, reps=1, n_queues=4):
    ntiles = (b_core + P - 1) // P

    nc = bacc.Bacc("TRN2", target_bir_lowering=False, num_swdge_queues=n_queues)
    feats = nc.dram_tensor(
        "features", [num_nodes, D], mybir.dt.float32, kind="ExternalInput"
    )
    # idx laid out host-side as [P, ntiles*S] int32: idx_t[p, j*S+s] = index of
    # sample s of batch row j*128+p (padded rows repeat the tile's first rows),
    # so each partition's indices are one contiguous run.
    idx = nc.dram_tensor(
        "idx_t", [P, ntiles * S], mybir.dt.int32, kind="ExternalInput"
    )
    # out padded to ntiles*P rows, host trims. out row j*128+p -> [p, j*D]
    out = nc.dram_tensor(
        "out", [ntiles * P, D], mybir.dt.float32, kind="ExternalOutput"
    )
    out_re = out[:].rearrange("(j p) d -> p j d", p=P)     # [P, ntiles, D]

    with tile.TileContext(nc) as tc:
        with (
            tc.tile_pool(name="idxp", bufs=1) as idxp,
            tc.tile_pool(name="gatp", bufs=3) as gatp,
            tc.tile_pool(name="resp", bufs=3) as resp,
        ):
            # all indices resident: [P, ntiles*S] int32 (6.3 KB/partition)
            idx_sb = idxp.tile([P, ntiles * S], mybir.dt.int32)
            nc.sync.dma_start(out=idx_sb[:], in_=idx[:])
            probe = idxp.tile([1, 1], mybir.dt.int32)
            nc.gpsimd.tensor_copy(probe[:], idx_sb[:1, :1])

            for j in [jj for _ in range(reps) for jj in range(ntiles)]:
                g = gatp.tile([P, S * D], mybir.dt.float32)
                # one Pool-engine touch absorbs the slot's WAR/WAW waits (the
                # SWDGE DMA struct accepts at most one sync wait in walrus).
                nc.gpsimd.memset(g[:1, :1], 0)
                for s in range(S):
                    inst = nc.gpsimd.indirect_dma_start(
                        out=g[:, s * D : (s + 1) * D],
                        out_offset=None,
                        in_=feats[:],
                        in_offset=bass.IndirectOffsetOnAxis(
                            ap=idx_sb[:, j * S + s : j * S + s + 1], axis=0
                        ),
                    )
                    if n_queues > 1:
                        # spread desc-gen across SWDGE queues (parallel Q7)
                        inst.ins.queue = f"qPoolDynamic{s % n_queues or ''}"
                # tree-reduce S=16 rows of D floats per partition
                width = S
                while width > 1:
                    half = width // 2
                    nc.vector.tensor_add(
                        g[:, 0 : half * D],
                        g[:, 0 : half * D],
                        g[:, half * D : 2 * half * D],
                    )
                    width = half
                r = resp.tile([P, D], mybir.dt.float32)
                nc.vector.tensor_scalar_mul(r[:], g[:, 0:D], 1.0 / S)
                nc.sync.dma_start(out=out_re[:, j, :], in_=r[:])
    nc.compile()
    return nc


_nc_cache = {}


def _get_nc(key):
    if key not in _nc_cache:
        _nc_cache[key] = build_nc(*key)
    return _nc_cache[key]


def _prep_idx(idx32):
    """[b_core, S] int32 -> [P, ntiles*S] partition-major layout (padded)."""
    b_core = idx32.shape[0]
    ntiles = (b_core + P - 1) // P
    pad = ntiles * P - b_core
    if pad:
        idx32 = np.concatenate([idx32, idx32[:pad]], axis=0)
    # [ntiles, P, S] -> [P, ntiles, S] -> [P, ntiles*S]
    return np.ascontiguousarray(
        idx32.reshape(ntiles, P, S).transpose(1, 0, 2).reshape(P, ntiles * S)
    )


def kernel(features, neigh_idx, num_sample):
    assert features.shape == (NUM_NODES, D)
    assert neigh_idx.shape == (BATCH, S)
    features = np.ascontiguousarray(features, dtype=np.float32)
    idx32 = np.asarray(neigh_idx).astype(np.int32)

    nc = _get_nc((NUM_NODES, B_CORE))
    in_maps = [
        {
            "features": features,
            "idx_t": _prep_idx(idx32[c * B_CORE : (c + 1) * B_CORE]),
        }
        for c in range(N_CORES)
    ]
    res = run_bass_kernel_spmd(nc, in_maps, core_ids=list(range(N_CORES)))
    return np.concatenate(
        [r["out"][:B_CORE] for r in res.results], axis=0
    )



# revision 3
# speedup vs baseline: 4.0062x; 4.0062x over previous
"""MeanAggregator (GNN mean message passing) on 8 Trainium2 NeuronCores.

reference:
    neigh_feats = features[neigh_idx]          # [batch, num_sample, d_feat]
    out = mean(neigh_feats, axis=1)            # [batch, d_feat]

Shapes (hardcoded): features [1_000_000, 128] f32, neigh_idx [100_000, 16] i64.

Strategy: data-parallel over the batch across 8 cores (12_500 rows each),
features replicated. Per core: per-partition indirect-DMA gathers (one
512 B feature row per partition per DMA — the only indirect form the SWDGE
ucode supports), vector-engine tree reduction over the 16 neighbors, scale
by 1/16, DMA out. Memory-bound: ~102 MB gathered + 6.4 MB written per core.

Layout per core: batch row b -> partition b % 128, slot b // 128.
Tile j covers batch rows [j*128, (j+1)*128): its 16 gathers (one per
neighbor sample) each fetch one row per partition into g[p, s*128:(s+1)*128].
"""

import numpy as np

import concourse.bacc as bacc
import concourse.bass as bass
import concourse.mybir as mybir
import concourse.tile as tile
from concourse.bass_utils import run_bass_kernel_spmd

N_CORES = 8
D = 128          # feature dim
S = 16           # neighbors per node
NUM_NODES = 1_000_000
BATCH = 100_000
B_CORE = BATCH // N_CORES   # 12_500
P = 128
NTILES = (B_CORE + P - 1) // P   # 98 (last tile ragged: 12500 = 97*128 + 84)


def build_nc(num_nodes=NUM_NODES, b_core=B_CORE, reps=1, n_queues=4):
    ntiles = (b_core + P - 1) // P

    nc = bacc.Bacc("TRN2", target_bir_lowering=False, num_swdge_queues=n_queues)
    feats = nc.dram_tensor(
        "features", [num_nodes, D], mybir.dt.float32, kind="ExternalInput"
    )
    # idx laid out host-side as [P, ntiles*S] int32: idx_t[p, j*S+s] = index of
    # sample s of batch row j*128+p (padded rows repeat the tile's first rows),
    # so each partition's indices are one contiguous run.
    idx = nc.dram_tensor(
        "idx_t", [P, ntiles * S], mybir.dt.int32, kind="ExternalInput"
    )
    # out padded to ntiles*P rows, host trims. out row j*128+p -> [p, j*D]
    out = nc.dram_tensor(
        "out", [ntiles * P, D], mybir.dt.float32, kind="ExternalOutput"
    )
    out_re = out[:].rearrange("(j p) d -> p j d", p=P)     # [P, ntiles, D]

    with tile.TileContext(nc) as tc:
        with (
            tc.tile_pool(name="idxp", bufs=1) as idxp,
            tc.tile_pool(name="gatp", bufs=3) as gatp,
            tc.tile_pool(name="resp", bufs=3) as resp,
        ):
            # all indices resident: [P, ntiles*S] int32 (6.3 KB/partition)
            idx_sb = idxp.tile([P, ntiles * S], mybir.dt.int32)
            nc.sync.dma_start(out=idx_sb[:], in_=idx[:])
            probe = idxp.tile([1, 1], mybir.dt.int32)
            nc.gpsimd.tensor_copy(probe[:], idx_sb[:1, :1])

            def tile_body(j):
                g = gatp.tile([P, S * D], mybir.dt.float32)
                # one Pool-engine touch absorbs the slot's WAR/WAW waits (the
                # SWDGE DMA struct accepts at most one sync wait in walrus).
                nc.gpsimd.memset(g[:1, :1], 0)
                for s in range(S):
                    inst = nc.gpsimd.indirect_dma_start(
                        out=g[:, s * D : (s + 1) * D],
                        out_offset=None,
                        in_=feats[:],
                        in_offset=bass.IndirectOffsetOnAxis(
                            ap=idx_sb[:, j * S + s : j * S + s + 1], axis=0
                        ),
                    )
                    if n_queues > 1:
                        # spread desc-gen across SWDGE queues (parallel Q7)
                        inst.ins.queue = f"qPoolDynamic{s % n_queues or ''}"
                # tree-reduce S=16 rows of D floats per partition
                width = S
                while width > 1:
                    half = width // 2
                    nc.vector.tensor_add(
                        g[:, 0 : half * D],
                        g[:, 0 : half * D],
                        g[:, half * D : 2 * half * D],
                    )
                    width = half
                r = resp.tile([P, D], mybir.dt.float32)
                nc.vector.tensor_scalar_mul(r[:], g[:, 0:D], 1.0 / S)
                nc.sync.dma_start(out=out_re[:, j, :], in_=r[:])

            if reps == 1:
                for j in range(ntiles):
                    tile_body(j)
            else:
                # repeat the whole kernel in a HW loop (timing harness only)
                with tc.For_i(0, reps, 1):
                    for j in range(ntiles):
                        tile_body(j)
    nc.compile()
    return nc


_nc_cache = {}


def _get_nc(key):
    if key not in _nc_cache:
        _nc_cache[key] = build_nc(*key)
    return _nc_cache[key]


def _prep_idx(idx32):
    """[b_core, S] int32 -> [P, ntiles*S] partition-major layout (padded)."""
    b_core = idx32.shape[0]
    ntiles = (b_core + P - 1) // P
    pad = ntiles * P - b_core
    if pad:
        idx32 = np.concatenate([idx32, idx32[:pad]], axis=0)
    # [ntiles, P, S] -> [P, ntiles, S] -> [P, ntiles*S]
    return np.ascontiguousarray(
        idx32.reshape(ntiles, P, S).transpose(1, 0, 2).reshape(P, ntiles * S)
    )


def kernel(features, neigh_idx, num_sample):
    assert features.shape == (NUM_NODES, D)
    assert neigh_idx.shape == (BATCH, S)
    features = np.ascontiguousarray(features, dtype=np.float32)
    idx32 = np.asarray(neigh_idx).astype(np.int32)

    nc = _get_nc((NUM_NODES, B_CORE))
    in_maps = [
        {
            "features": features,
            "idx_t": _prep_idx(idx32[c * B_CORE : (c + 1) * B_CORE]),
        }
        for c in range(N_CORES)
    ]
    res = run_bass_kernel_spmd(nc, in_maps, core_ids=list(range(N_CORES)))
    return np.concatenate(
        [r["out"][:B_CORE] for r in res.results], axis=0
    )

